# revision 4
# baseline (speedup 1.0000x reference)
"""Sparse 3D max-pool (kernel=stride=2) on 8 trn2 NeuronCores.

Strategy (per the "pre-sort voxels by coarse id" sharding option):
  Host: compute coarse segment id per point; core c owns segments
  [c*32768, (c+1)*32768). Per core, order its segments by descending
  point count ("rank"), and lay the core's feature rows out as
  "rounds": round j holds the j-th point of every segment that has
  one.  Because ranks are count-descending, round j covers a prefix
  of ranks, so the device sees fixed rectangular tiles.
  Device: stream round 0 contiguously into an SBUF accumulator tile,
  then tensor_tensor(max) each later round into the accumulator
  prefix; write finished blocks to DRAM.  No indirect DMA anywhere.
  Host: un-permute per-core results into the full [S, C] output.

Empty segments read a zeros sentinel row in round 0 (reference maps
-inf -> 0 for uncovered voxels); round j>=1 padding reads a -inf
sentinel row (identity for max).
"""

import numpy as np

import concourse.bass as bass
import concourse.tile as tile
from concourse import bacc, mybir
from concourse.bass_utils import run_bass_kernel_spmd

P = 128          # SBUF partitions
C = 64           # channels
POOL = 2
COARSE = 64      # coarse resolution per axis
S_TOT = COARSE ** 3          # 262144 coarse segments
N_CORES = 8
S_CORE = S_TOT // N_CORES    # 32768 segments per core
N_BLOCKS = 8
S_BLK = S_CORE // N_BLOCKS   # 8192 segments per block
G_BLK = S_BLK // P           # 64 free-dim groups per block

_prog_cache = {}


def _build_program(round_plan, r_total, repeat=1, num_devices=N_CORES, bufs=(2, 6)):
    """round_plan: list over blocks of list of padded round lengths
    (rows, multiples of 128; round 0 is always S_BLK)."""
    nc = bacc.Bacc(
        "TRN2", target_bir_lowering=False, debug=False, num_devices=num_devices
    )
    xs = nc.dram_tensor(
        "xs", [r_total, C], mybir.dt.float32, kind="ExternalInput"
    ).ap()
    out = nc.dram_tensor(
        "out", [S_CORE, C], mybir.dt.float32, kind="ExternalOutput"
    ).ap()

    with tile.TileContext(nc) as tc:
        with (
            tc.tile_pool(name="acc", bufs=bufs[0]) as acc_pool,
            tc.tile_pool(name="ld", bufs=bufs[1]) as ld_pool,
        ):
            for _ in range(repeat):
                r0 = 0
                for b, rounds in enumerate(round_plan):
                    acc = acc_pool.tile([P, G_BLK * C], mybir.dt.float32, tag="acc")
                    for j, L in enumerate(rounds):
                        g = L // P
                        src = xs[r0 : r0 + L, :].rearrange(
                            "(p x) c -> p (x c)", p=P
                        )
                        if j == 0:
                            assert L == S_BLK
                            nc.sync.dma_start(acc[:], src)
                        else:
                            buf = ld_pool.tile([P, g * C], mybir.dt.float32, tag="buf")
                            nc.sync.dma_start(buf[:], src)
                            nc.vector.tensor_tensor(
                                out=acc[:, : g * C],
                                in0=acc[:, : g * C],
                                in1=buf[:],
                                op=mybir.AluOpType.max,
                            )
                        r0 += L
                    dst = out[b * S_BLK : (b + 1) * S_BLK, :].rearrange(
                        "(p x) c -> p (x c)", p=P
                    )
                    nc.sync.dma_start(dst, acc[:])
                assert r0 == r_total
    nc.compile()
    return nc


def _ceil128(x):
    return (int(x) + P - 1) // P * P


def kernel(features, coords, num_segments):
    features = np.ascontiguousarray(features, dtype=np.float32)
    coords = np.asarray(coords)
    N = features.shape[0]
    assert int(num_segments) == S_TOT and features.shape[1] == C

    seg = (
        (coords[:, 0].astype(np.int64) >> 1) * (COARSE * COARSE)
        + (coords[:, 1].astype(np.int64) >> 1) * COARSE
        + (coords[:, 2].astype(np.int64) >> 1)
    ).astype(np.int32)

    counts_all = np.bincount(seg, minlength=S_TOT)
    order = np.argsort(seg, kind="stable").astype(np.int64)
    starts_all = np.zeros(S_TOT + 1, dtype=np.int64)
    np.cumsum(counts_all, out=starts_all[1:])

    ZERO_ROW = np.int64(N)      # sentinel: zeros row
    NEG_ROW = np.int64(N + 1)   # sentinel: -inf row

    # per-core metadata
    core_rank_order = []   # local seg idx sorted by count desc
    core_sorted_cnt = []
    for c in range(N_CORES):
        cnt = counts_all[c * S_CORE : (c + 1) * S_CORE]
        ro = np.argsort(-cnt, kind="stable")
        core_rank_order.append(ro)
        core_sorted_cnt.append(cnt[ro])

    J = int(max(sc[0] for sc in core_sorted_cnt))  # max count anywhere

    # M[c][r, j] = feature row of the j-th point of rank-r segment (or sentinel)
    js = np.arange(J, dtype=np.int64)
    Ms = []
    for c in range(N_CORES):
        sc = core_sorted_cnt[c]
        st = starts_all[c * S_CORE + core_rank_order[c]]
        flat = st[:, None] + js[None, :]
        valid = js[None, :] < sc[:, None]
        M = np.where(valid, order[np.minimum(flat, N - 1)], NEG_ROW)
        M[:, 0][~valid[:, 0]] = ZERO_ROW
        Ms.append(M)

    # uniform padded round lengths: L[c][b][j] -> max over cores, pad to 128
    # S_j per core = #segments with count > j  (j>=1); round 0 full block.
    round_plan = []  # [block][j] = padded rows
    for b in range(N_BLOCKS):
        lens = [S_BLK]
        for j in range(1, J):
            mx = 0
            for c in range(N_CORES):
                s_j = int(np.count_nonzero(core_sorted_cnt[c] > j))
                mx = max(mx, min(s_j, (b + 1) * S_BLK) - b * S_BLK)
            if mx <= 0:
                break
            lens.append(_ceil128(mx))
        round_plan.append(lens)
    r_total = sum(sum(r) for r in round_plan)

    # build per-core gather index vectors in device layout order
    in_maps = []
    feats_ext = np.concatenate(
        [
            features,
            np.zeros((1, C), np.float32),
            np.full((1, C), -np.inf, np.float32),
        ],
        axis=0,
    )
    for c in range(N_CORES):
        M = Ms[c]
        parts = []
        for b, rounds in enumerate(round_plan):
            for j, L in enumerate(rounds):
                g = L // P
                col = M[b * S_BLK : b * S_BLK + L, j]
                # rank-local r = g*128 + p  ->  pos = p*g_cnt + g
                parts.append(col.reshape(g, P).T.ravel())
        idx = np.concatenate(parts)
        assert idx.shape[0] == r_total
        xs = feats_ext[idx]
        in_maps.append({"xs": np.ascontiguousarray(xs)})

    key = (r_total, tuple(tuple(r) for r in round_plan))
    if key not in _prog_cache:
        _prog_cache[key] = _build_program(round_plan, r_total)
    nc = _prog_cache[key]

    res = run_bass_kernel_spmd(nc, in_maps, core_ids=list(range(N_CORES)))

    out_full = np.empty((S_TOT, C), dtype=np.float32)
    for c in range(N_CORES):
        rows = res.results[c]["out"]  # [S_CORE, C] in device pos order
        ro = core_rank_order[c]
        seg_for_pos = np.concatenate(
            [
                ro[b * S_BLK : (b + 1) * S_BLK].reshape(G_BLK, P).T.ravel()
                for b in range(N_BLOCKS)
            ]
        )
        out_full[c * S_CORE + seg_for_pos] = rows
    return out_full
